# revision 9
# baseline (speedup 1.0000x reference)
import sys

sys.path.insert(0, "/opt/trn_rl_repo")
import numpy as np

B, S, M, D = 16, 2048, 128, 1024
HEADS, DH = 16, 64
NCORES = 8
BPC = B // NCORES  # batches per core
DC = 8  # 128-row chunks of D
EPS = 1e-5
SCALE = DH ** -0.5

_state = {}


def _build():
    import concourse.bass as bass
    import concourse.bacc as bacc
    import concourse.tile as tile
    from concourse.masks import make_identity

    mybir = bass.mybir
    dt = mybir.dt
    f32, f32r, bf16 = dt.float32, dt.float32r, dt.bfloat16
    AF = mybir.ActivationFunctionType

    nc = bacc.Bacc("TRN2", target_bir_lowering=False)
    xnt_d = nc.declare_dram_parameter("xnt", [BPC, D, S], f32r, isOutput=False)
    lnt_d = nc.declare_dram_parameter("lnt", [D, BPC * M], f32r, isOutput=False)
    wq_d = nc.declare_dram_parameter("wq", [D, D], f32r, isOutput=False)
    wk_d = nc.declare_dram_parameter("wk", [D, D], f32r, isOutput=False)
    wv_d = nc.declare_dram_parameter("wv", [D, D], f32r, isOutput=False)
    wo_d = nc.declare_dram_parameter("wo", [D, D], f32r, isOutput=False)
    out_d = nc.declare_dram_parameter("out", [BPC, M, D], f32, isOutput=True)

    xr = xnt_d.rearrange("b (c p) j -> b p c j", p=128)

    with tile.TileContext(nc) as tc:
        with (
            tc.tile_pool(name="pw1", bufs=2) as pw1,
            tc.tile_pool(name="pq", bufs=1) as pq,
            tc.tile_pool(name="plnt", bufs=1) as plnt,
            tc.tile_pool(name="px", bufs=2) as px,
            tc.tile_pool(name="pv", bufs=1) as pv,
            tc.tile_pool(name="pexp", bufs=2) as pexp,
            tc.tile_pool(name="psmall", bufs=1) as psmall,
            tc.tile_pool(name="pavn", bufs=1) as pavn,
            tc.tile_pool(name="pavt", bufs=2) as pavt,
            tc.tile_pool(name="pout", bufs=2) as pout,
            tc.tile_pool(name="prr", bufs=4) as prr,
            tc.tile_pool(name="ps256", bufs=2, space="PSUM") as ps256,
            tc.tile_pool(name="ps512", bufs=2, space="PSUM") as ps512,
            tc.tile_pool(name="pssim", bufs=2, space="PSUM") as pssim,
            tc.tile_pool(name="psav", bufs=2, space="PSUM") as psav,
        ):
            wk_t = pw1.tile([128, DC, D], f32r, tag="w")
            nc.sync.dma_start(wk_t[:], wk_d.rearrange("(c p) f -> p c f", p=128))
            wv_t = pw1.tile([128, DC, D], f32r, tag="w")
            nc.sync.dma_start(wv_t[:], wv_d.rearrange("(c p) f -> p c f", p=128))
            wq_t = pq.tile([128, DC, D], f32r, tag="q")
            nc.sync.dma_start(wq_t[:], wq_d.rearrange("(c p) f -> p c f", p=128))
            lnt_t = plnt.tile([128, DC, BPC * M], f32r)
            nc.sync.dma_start(lnt_t[:], lnt_d.rearrange("(c p) m -> p c m", p=128))

            ident = psmall.tile([128, 128], f32, tag="id")
            make_identity(nc, ident[:])

            # ---- q phase: qt[f, m] = (scale*Wq) @ lnT, bf16 ----
            qt = psmall.tile([128, 8, BPC * M], bf16, tag="qt")
            for ft in range(8):
                qps = ps256.tile([128, BPC * M], f32, tag="t256")
                for c in range(DC):
                    nc.tensor.matmul(
                        qps[:],
                        wq_t[:, c, ft * 128 : (ft + 1) * 128],
                        lnt_t[:, c, :],
                        start=(c == 0),
                        stop=(c == DC - 1),
                    )
                nc.scalar.copy(qt[:, ft, :], qps[:])

            avts = []
            for b in range(BPC):
                # ---- kv phase: kT[f, j] and v[j, f] for this batch ----
                kt = pq.tile([128, 8, 16, 128], bf16, tag="q")
                vt = pv.tile([128, 16, HEADS, DH + 1], bf16)
                nc.gpsimd.memset(vt[:, :, :, DH : DH + 1], 1.0)
                for s8 in range(8):
                    xs = px.tile([128, DC, 256], f32r)
                    nc.sync.dma_start(xs[:], xr[b, :, :, s8 * 256 : (s8 + 1) * 256])
                    for ft in range(8):
                        kps = ps256.tile([128, 256], f32, tag="t256")
                        for c in range(DC):
                            nc.tensor.matmul(
                                kps[:],
                                wk_t[:, c, ft * 128 : (ft + 1) * 128],
                                xs[:, c, :],
                                start=(c == 0),
                                stop=(c == DC - 1),
                            )
                        nc.scalar.copy(kt[:, ft, 2 * s8 : 2 * s8 + 2, :], kps[:])
                    for jl in range(2):
                        jt = 2 * s8 + jl
                        for fs in range(2):
                            vps = ps512.tile([128, 512], f32, tag="t512")
                            for c in range(DC):
                                nc.tensor.matmul(
                                    vps[:],
                                    xs[:, c, jl * 128 : (jl + 1) * 128],
                                    wv_t[:, c, fs * 512 : (fs + 1) * 512],
                                    start=(c == 0),
                                    stop=(c == DC - 1),
                                )
                            nc.scalar.copy(
                                vt[:, jt, fs * 8 : (fs + 1) * 8, 0:DH], vps[:]
                            )

                # ---- attention ----
                avn = pavn.tile([128, 8, 128], f32)
                for h in range(HEADS):
                    hp, hr = h // 2, (h % 2) * 64
                    ex = pexp.tile([128, 16, 128], bf16)
                    for jq in range(4):
                        sps = pssim.tile([128, 4, 128], f32, tag="s")
                        for jl in range(4):
                            nc.tensor.matmul(
                                sps[:, jl, :],
                                kt[hr : hr + 64, hp, 4 * jq + jl, :],
                                qt[hr : hr + 64, hp, b * M : (b + 1) * M],
                                start=True,
                                stop=True,
                            )
                        nc.scalar.activation(
                            ex[:, 4 * jq : 4 * jq + 4, :], sps[:]
                        , AF.Exp)
                    aps = psav.tile([128, DH + 1], f32)
                    for jt in range(16):
                        nc.tensor.matmul(
                            aps[:],
                            ex[:, jt, :],
                            vt[:, jt, h, :],
                            start=(jt == 0),
                            stop=(jt == 15),
                        )
                    rr = prr.tile([128, 1], f32)
                    nc.vector.reciprocal(rr[:], aps[:, DH : DH + 1])
                    nc.scalar.activation(
                        avn[:, hp, hr : hr + 64], aps[:, 0:DH], AF.Copy, scale=rr[:]
                    )
                # transpose avn (m, i) -> avt (i, m)
                avt = pavt.tile([128, 8, 128], f32r)
                for ic in range(8):
                    tps = pssim.tile([128, 128], f32, tag="s")
                    nc.tensor.transpose(tps[:], avn[:, ic, :], ident[:])
                    nc.scalar.copy(avt[:, ic, :], tps[:])
                avts.append(avt)

            # ---- out = avT.T @ WoT ----
            wo_t = pw1.tile([128, DC, D], f32r, tag="w")
            nc.sync.dma_start(wo_t[:], wo_d.rearrange("(c p) f -> p c f", p=128))
            for b in range(BPC):
                osb = pout.tile([128, D], f32)
                for es in range(2):
                    ops = ps512.tile([128, 512], f32, tag="t512")
                    for ic in range(8):
                        nc.tensor.matmul(
                            ops[:],
                            avts[b][:, ic, :],
                            wo_t[:, ic, es * 512 : (es + 1) * 512],
                            start=(ic == 0),
                            stop=(ic == 7),
                        )
                    nc.scalar.copy(osb[:, es * 512 : (es + 1) * 512], ops[:])
                nc.sync.dma_start(out_d[b], osb[:])

    nc.compile()
    return nc


def _ln(a, g, bt):
    a64 = a.astype(np.float64)
    mu = a64.mean(-1, keepdims=True)
    var = ((a64 - mu) ** 2).mean(-1, keepdims=True)
    n = (a64 - mu) / np.sqrt(var + EPS)
    return (n * g.astype(np.float64) + bt.astype(np.float64)).astype(np.float32)


def _prep(x, latents, gx, bx, gl, bl, Wq, Wkv, Wo):
    xn = _ln(x, gx, bx)
    ln = _ln(latents, gl, bl)
    wq_h = np.ascontiguousarray((SCALE * Wq).T.astype(np.float32))
    wk_h = np.ascontiguousarray(Wkv[:D].T.astype(np.float32))
    wv_h = np.ascontiguousarray(Wkv[D:].T.astype(np.float32))
    wo_h = np.ascontiguousarray(Wo.T.astype(np.float32))
    per_core = []
    for c in range(NCORES):
        xnt = np.ascontiguousarray(xn[c * BPC : (c + 1) * BPC].transpose(0, 2, 1))
        lnt = np.ascontiguousarray(
            np.concatenate(
                [ln[c * BPC + i].T for i in range(BPC)], axis=1
            )
        )
        per_core.append(
            {
                "xnt": xnt,
                "lnt": lnt,
                "wq": wq_h,
                "wk": wk_h,
                "wv": wv_h,
                "wo": wo_h,
            }
        )
    return per_core


def kernel(x, latents, gx, bx, gl, bl, Wq, Wkv, Wo):
    from concourse.bass_utils import run_bass_kernel_spmd

    x, latents = np.asarray(x), np.asarray(latents)
    gx, bx, gl, bl = map(np.asarray, (gx, bx, gl, bl))
    Wq, Wkv, Wo = np.asarray(Wq), np.asarray(Wkv), np.asarray(Wo)
    if "nc" not in _state:
        _state["nc"] = _build()
    nc = _state["nc"]
    per_core = _prep(x, latents, gx, bx, gl, bl, Wq, Wkv, Wo)
    res = run_bass_kernel_spmd(nc, per_core, core_ids=list(range(NCORES)))
    _state["last"] = (per_core, res)
    out = np.concatenate(
        [np.asarray(res.results[c]["out"]) for c in range(NCORES)], axis=0
    )
    return out.astype(np.float32)


# revision 19
# speedup vs baseline: 891002.0000x; 891002.0000x over previous
import sys

sys.path.insert(0, "/opt/trn_rl_repo")
import numpy as np

B, S, M, D = 16, 2048, 128, 1024
HEADS, DH = 16, 64
NCORES = 8
BPC = B // NCORES  # batches per core
DC = 8  # 128-row chunks of D
EPS = 1e-5
SCALE = DH ** -0.5

_state = {}


def _build():
    import concourse.bass as bass
    import concourse.bacc as bacc
    import concourse.tile as tile
    from concourse.masks import make_identity

    mybir = bass.mybir
    dt = mybir.dt
    f32, f32r, bf16 = dt.float32, dt.float32r, dt.bfloat16
    AF = mybir.ActivationFunctionType

    nc = bacc.Bacc("TRN2", target_bir_lowering=False)
    xnt_d = nc.declare_dram_parameter("xnt", [BPC, D, S], f32r, isOutput=False)
    lnt_d = nc.declare_dram_parameter("lnt", [D, BPC * M], f32r, isOutput=False)
    wq_d = nc.declare_dram_parameter("wq", [D, D], f32r, isOutput=False)
    wk_d = nc.declare_dram_parameter("wk", [D, D], f32r, isOutput=False)
    wv_d = nc.declare_dram_parameter("wv", [D, D], f32r, isOutput=False)
    wo_d = nc.declare_dram_parameter("wo", [D, D], f32r, isOutput=False)
    out_d = nc.declare_dram_parameter("out", [BPC, M, D], f32, isOutput=True)

    xr = xnt_d.rearrange("b (c p) j -> b p c j", p=128)

    with tile.TileContext(nc) as tc:
        with (
            tc.tile_pool(name="pw1", bufs=2) as pw1,
            tc.tile_pool(name="pq", bufs=1) as pq,
            tc.tile_pool(name="plnt", bufs=1) as plnt,
            tc.tile_pool(name="px", bufs=2) as px,
            tc.tile_pool(name="pv", bufs=1) as pv,
            tc.tile_pool(name="pexp", bufs=2) as pexp,
            tc.tile_pool(name="psmall", bufs=1) as psmall,
            tc.tile_pool(name="pavn", bufs=1) as pavn,
            tc.tile_pool(name="pavt", bufs=2) as pavt,
            tc.tile_pool(name="pout", bufs=2) as pout,
            tc.tile_pool(name="prr", bufs=4) as prr,
            tc.tile_pool(name="ps256", bufs=2, space="PSUM") as ps256,
            tc.tile_pool(name="ps512", bufs=2, space="PSUM") as ps512,
            tc.tile_pool(name="pssim", bufs=2, space="PSUM") as pssim,
            tc.tile_pool(name="psav", bufs=2, space="PSUM") as psav,
        ):
            wk_t = pw1.tile([128, DC, D], f32r, tag="w")
            nc.sync.dma_start(wk_t[:], wk_d.rearrange("(c p) f -> p c f", p=128))
            wv_t = pw1.tile([128, DC, D], f32r, tag="w")
            nc.sync.dma_start(wv_t[:], wv_d.rearrange("(c p) f -> p c f", p=128))
            wq_t = pq.tile([128, DC, D], f32r, tag="q")
            nc.sync.dma_start(wq_t[:], wq_d.rearrange("(c p) f -> p c f", p=128))
            lnt_t = plnt.tile([128, DC, BPC * M], f32r)
            nc.sync.dma_start(lnt_t[:], lnt_d.rearrange("(c p) m -> p c m", p=128))

            ident = psmall.tile([128, 128], f32, tag="id")
            make_identity(nc, ident[:])

            # ---- q phase: qt[f, m] = (scale*Wq) @ lnT, bf16 ----
            qt = psmall.tile([128, 8, BPC * M], bf16, tag="qt")
            for ft in range(8):
                qps = ps256.tile([128, BPC * M], f32, tag="t256")
                for c in range(DC):
                    nc.tensor.matmul(
                        qps[:],
                        wq_t[:, c, ft * 128 : (ft + 1) * 128],
                        lnt_t[:, c, :],
                        start=(c == 0),
                        stop=(c == DC - 1),
                    )
                nc.scalar.copy(qt[:, ft, :], qps[:])

            avts = []
            for b in range(BPC):
                # ---- kv phase: kT[f, j] and v[j, f] for this batch ----
                kt = pq.tile([128, 8, 16, 128], bf16, tag="q")
                vt = pv.tile([128, 16, HEADS, DH + 1], bf16)
                nc.gpsimd.memset(vt[:, :, :, DH : DH + 1], 1.0)
                for s8 in range(8):
                    xs = px.tile([128, DC, 256], f32r)
                    nc.sync.dma_start(xs[:], xr[b, :, :, s8 * 256 : (s8 + 1) * 256])
                    for ft in range(8):
                        kps = ps256.tile([128, 256], f32, tag="t256")
                        for c in range(DC):
                            nc.tensor.matmul(
                                kps[:],
                                wk_t[:, c, ft * 128 : (ft + 1) * 128],
                                xs[:, c, :],
                                start=(c == 0),
                                stop=(c == DC - 1),
                            )
                        nc.scalar.copy(kt[:, ft, 2 * s8 : 2 * s8 + 2, :], kps[:])
                    for jl in range(2):
                        jt = 2 * s8 + jl
                        for fs in range(2):
                            vps = ps512.tile([128, 512], f32, tag="t512")
                            for c in range(DC):
                                nc.tensor.matmul(
                                    vps[:],
                                    xs[:, c, jl * 128 : (jl + 1) * 128],
                                    wv_t[:, c, fs * 512 : (fs + 1) * 512],
                                    start=(c == 0),
                                    stop=(c == DC - 1),
                                )
                            nc.scalar.copy(
                                vt[:, jt, fs * 8 : (fs + 1) * 8, 0:DH], vps[:]
                            )

                # ---- attention ----
                avn = pavn.tile([128, 8, 128], f32)
                for h in range(HEADS):
                    hp, hr = h // 2, (h % 2) * 64
                    ex = pexp.tile([128, 16, 128], bf16)
                    for jq in range(4):
                        sps = pssim.tile([128, 4, 128], f32, tag="s")
                        for jl in range(4):
                            nc.tensor.matmul(
                                sps[:, jl, :],
                                kt[hr : hr + 64, hp, 4 * jq + jl, :],
                                qt[hr : hr + 64, hp, b * M : (b + 1) * M],
                                start=True,
                                stop=True,
                            )
                        nc.scalar.activation(
                            ex[:, 4 * jq : 4 * jq + 4, :], sps[:]
                        , AF.Exp)
                    aps = psav.tile([128, DH + 1], f32)
                    for jt in range(16):
                        nc.tensor.matmul(
                            aps[:],
                            ex[:, jt, :],
                            vt[:, jt, h, :],
                            start=(jt == 0),
                            stop=(jt == 15),
                        )
                    rr = prr.tile([128, 1], f32)
                    nc.vector.reciprocal(rr[:], aps[:, DH : DH + 1])
                    nc.scalar.activation(
                        avn[:, hp, hr : hr + 64], aps[:, 0:DH], AF.Copy, scale=rr[:]
                    )
                # transpose avn (m, i) -> avt (i, m)
                avt = pavt.tile([128, 8, 128], f32r)
                for ic in range(8):
                    tps = pssim.tile([128, 128], f32, tag="s")
                    nc.tensor.transpose(tps[:], avn[:, ic, :], ident[:])
                    nc.scalar.copy(avt[:, ic, :], tps[:])
                avts.append(avt)

            # ---- out = avT.T @ WoT ----
            wo_t = pw1.tile([128, DC, D], f32r, tag="w")
            nc.sync.dma_start(wo_t[:], wo_d.rearrange("(c p) f -> p c f", p=128))
            for b in range(BPC):
                osb = pout.tile([128, D], f32)
                for es in range(2):
                    ops = ps512.tile([128, 512], f32, tag="t512")
                    for ic in range(8):
                        nc.tensor.matmul(
                            ops[:],
                            avts[b][:, ic, :],
                            wo_t[:, ic, es * 512 : (es + 1) * 512],
                            start=(ic == 0),
                            stop=(ic == 7),
                        )
                    nc.scalar.copy(osb[:, es * 512 : (es + 1) * 512], ops[:])
                nc.sync.dma_start(out_d[b], osb[:])

    nc.compile()
    return nc


def _ln(a, g, bt):
    a64 = a.astype(np.float64)
    mu = a64.mean(-1, keepdims=True)
    var = ((a64 - mu) ** 2).mean(-1, keepdims=True)
    n = (a64 - mu) / np.sqrt(var + EPS)
    return (n * g.astype(np.float64) + bt.astype(np.float64)).astype(np.float32)


def _prep(x, latents, gx, bx, gl, bl, Wq, Wkv, Wo):
    xn = _ln(x, gx, bx)
    ln = _ln(latents, gl, bl)
    wq_h = np.ascontiguousarray((SCALE * Wq).T.astype(np.float32))
    wk_h = np.ascontiguousarray(Wkv[:D].T.astype(np.float32))
    wv_h = np.ascontiguousarray(Wkv[D:].T.astype(np.float32))
    wo_h = np.ascontiguousarray(Wo.T.astype(np.float32))
    per_core = []
    for c in range(NCORES):
        xnt = np.ascontiguousarray(xn[c * BPC : (c + 1) * BPC].transpose(0, 2, 1))
        lnt = np.ascontiguousarray(
            np.concatenate(
                [ln[c * BPC + i].T for i in range(BPC)], axis=1
            )
        )
        per_core.append(
            {
                "xnt": xnt,
                "lnt": lnt,
                "wq": wq_h,
                "wk": wk_h,
                "wv": wv_h,
                "wo": wo_h,
            }
        )
    return per_core


def kernel(x, latents, gx, bx, gl, bl, Wq, Wkv, Wo):
    from concourse.bass_utils import run_bass_kernel_spmd

    x, latents = np.asarray(x), np.asarray(latents)
    gx, bx, gl, bl = map(np.asarray, (gx, bx, gl, bl))
    Wq, Wkv, Wo = np.asarray(Wq), np.asarray(Wkv), np.asarray(Wo)
    if "nc" not in _state:
        _state["nc"] = _build()
    nc = _state["nc"]
    per_core = _prep(x, latents, gx, bx, gl, bl, Wq, Wkv, Wo)
    res = run_bass_kernel_spmd(nc, per_core, core_ids=list(range(NCORES)))
    _state["last"] = (per_core, res)
    out = np.concatenate(
        [np.asarray(res.results[c]["out"]) for c in range(NCORES)], axis=0
    )
    return out.astype(np.float32)
